# revision 1
# baseline (speedup 1.0000x reference)
"""MeshGraphDecoder Trainium2 kernel (8-core SPMD).

Sharding: grid nodes (and their incoming edges) are partitioned across 8
cores; mesh features and MLP weights are replicated (gathered on host
into per-edge streams). Within a core, nodes are packed into 256
windows of 128 nodes such that each window's incoming-edge count fits a
global per-window capacity schedule (T_w*128 slots, T_w in {3,4}); the
schedule is shared by all cores so one SPMD program serves all 8.

Device pipeline per window w (ET = 128*T_w edges):
  edge MLP   : catT [384, ET] chunks -> 6 matmuls -> H [256, ET] PSUM
               -> SiLU+b1 (ACT) -> per-128-edge tile: 2 matmuls ->
               ef2 [128e,128d] -> +b2, LayerNorm (DVE)
  aggregate  : onehotT[e,n] = (slot[e] == iota[n]) ; aggT [128d,128n]
               += ef2^T @ onehotT  (PSUM-accumulated over T_w tiles)
  node MLP   : per 4 windows (512 nodes): cat(aggT, gridT) -> 4 matmuls
               -> SiLU+b1 -> per-128-node tile: 2 matmuls -> +b2,
               LayerNorm, +grid residual -> out rows
"""

import numpy as np

N_MESH = 40962
N_GRID = 262144
N_EDGE = 786432
D = 128
HID = 256
EPS = 1e-5
N_CORES = 8
N_SH = N_GRID // N_CORES
W_PER_CORE = N_SH // 128
P = 128
SW = 4  # windows per node-stage supertile


# ----------------------------------------------------------------- host prep

def _pack_core(degrees, caps):
    n = len(degrees)
    n_win = len(caps)
    caps = np.asarray(caps, dtype=np.int64)
    order = np.argsort(-degrees, kind="stable")
    sums = np.zeros(n_win, dtype=np.int64)
    cnts = np.zeros(n_win, dtype=np.int64)
    assign = np.empty(n, dtype=np.int64)
    mean = degrees.sum() / n
    caps_f = caps.astype(np.float64)
    for nid in order:
        slack = (caps_f - sums) - mean * (128 - cnts)
        slack[cnts >= 128] = -np.inf
        w = int(np.argmax(slack))
        assign[nid] = w
        sums[w] += degrees[nid]
        cnts[w] += 1
    members = [list(np.nonzero(assign == w)[0]) for w in range(n_win)]
    for _ in range(200000):
        over = np.nonzero(sums > caps)[0]
        if len(over) == 0:
            break
        w = int(over[0])
        mw = members[w]
        a = max(mw, key=lambda i: degrees[i])
        v = int(np.argmax(caps - sums))
        mv = members[v]
        b = min(mv, key=lambda i: degrees[i])
        da, db = int(degrees[a]), int(degrees[b])
        assert da > db and (caps[v] - sums[v]) >= (da - db), "repair stuck"
        mw.remove(a); mv.remove(b)
        mw.append(b); mv.append(a)
        sums[w] += db - da
        sums[v] += da - db
    else:
        raise RuntimeError("window repair did not converge")
    perm = np.empty(n, dtype=np.int64)
    for w in range(n_win):
        perm[w * 128 : (w + 1) * 128] = members[w]
    return perm


def _prepare(inputs):
    dst = np.asarray(inputs["dst_idx"]).astype(np.int64)
    src = np.asarray(inputs["src_idx"]).astype(np.int64)
    ef = np.asarray(inputs["m2g_efeat"], dtype=np.float32)
    gf = np.asarray(inputs["grid_nfeat"], dtype=np.float32)
    mf = np.asarray(inputs["mesh_nfeat"], dtype=np.float32)

    core_of_edge = dst // N_SH
    e_counts = np.bincount(core_of_edge, minlength=N_CORES)
    base = W_PER_CORE * 3 * 128
    a = max(0, (int(e_counts.max()) - base + 127) // 128) + 8
    T_seq = np.array([4] * a + [3] * (W_PER_CORE - a), dtype=np.int64)
    caps = T_seq * 128
    C = int(caps.sum())
    win_off = np.concatenate([[0], np.cumsum(caps)])[:-1]

    cores = []
    unperm = np.empty(N_GRID, dtype=np.int64)
    for c in range(N_CORES):
        lo = c * N_SH
        mask = core_of_edge == c
        deg = np.bincount(dst[mask] - lo, minlength=N_SH)
        perm = _pack_core(deg, caps)
        inv = np.empty(N_SH, dtype=np.int64)
        inv[perm] = np.arange(N_SH)

        e_ids = np.nonzero(mask)[0]
        wslot = inv[dst[e_ids] - lo]
        w = wslot >> 7
        slot = wslot & 127
        order = np.lexsort((slot, w))
        e_ids, w, slot = e_ids[order], w[order], slot[order]
        cnt = np.bincount(w, minlength=W_PER_CORE)
        assert (cnt <= caps).all()
        within = np.arange(len(e_ids)) - np.repeat(
            np.concatenate([[0], np.cumsum(cnt)])[:-1], cnt
        )
        pos = win_off[w] + within

        dlf = np.full(C, -1.0, dtype=np.float32)
        dlf[pos] = slot.astype(np.float32)
        e0 = np.zeros((C, D), np.float32)
        e1 = np.zeros((C, D), np.float32)
        e2 = np.zeros((C, D), np.float32)
        e0[pos] = ef[e_ids]
        e1[pos] = mf[src[e_ids]]
        e2[pos] = gf[dst[e_ids]]

        gperm = perm + lo
        cores.append(
            dict(
                e0T=np.ascontiguousarray(e0.T),
                e1T=np.ascontiguousarray(e1.T),
                e2T=np.ascontiguousarray(e2.T),
                dlf=dlf,
                gridT=np.ascontiguousarray(gf[gperm].T),
                grid_res=np.ascontiguousarray(gf[gperm]),
            )
        )
        unperm[gperm] = c * N_SH + np.arange(N_SH)
    return T_seq, C, cores, unperm


# ------------------------------------------------------------- device program

def _build_program(T_seq, C, trivial_eln, trivial_nln, limit_windows=None):
    import concourse.bass as bass
    import concourse.tile as tile
    from concourse import bacc, mybir

    f32 = mybir.dt.float32
    f32r = mybir.dt.float32r
    AF = mybir.ActivationFunctionType
    OP = mybir.AluOpType

    nc = bacc.Bacc("TRN2", target_bir_lowering=False)

    e0T = nc.dram_tensor("e0T", [P, C], f32r, kind="ExternalInput")
    e1T = nc.dram_tensor("e1T", [P, C], f32r, kind="ExternalInput")
    e2T = nc.dram_tensor("e2T", [P, C], f32r, kind="ExternalInput")
    dlf = nc.dram_tensor("dlf", [C], f32, kind="ExternalInput")
    gridT = nc.dram_tensor("gridT", [P, N_SH], f32, kind="ExternalInput")
    grid_res = nc.dram_tensor("grid_res", [N_SH, D], f32, kind="ExternalInput")
    ew1 = nc.dram_tensor("ew1", [3 * D, HID], f32r, kind="ExternalInput")
    ew2 = nc.dram_tensor("ew2", [HID, D], f32, kind="ExternalInput")
    nw1 = nc.dram_tensor("nw1", [2 * D, HID], f32, kind="ExternalInput")
    nw2 = nc.dram_tensor("nw2", [HID, D], f32, kind="ExternalInput")
    eb1 = nc.dram_tensor("eb1", [HID], f32, kind="ExternalInput")
    nb1 = nc.dram_tensor("nb1", [HID], f32, kind="ExternalInput")
    eb2r = nc.dram_tensor("eb2r", [P, D], f32, kind="ExternalInput")
    nb2r = nc.dram_tensor("nb2r", [P, D], f32, kind="ExternalInput")
    iota = nc.dram_tensor("iota", [P, P], f32, kind="ExternalInput")
    # general-LN scale/shift (replicated rows); loaded only if nontrivial
    egr = nc.dram_tensor("egr", [P, D], f32, kind="ExternalInput")
    ebr = nc.dram_tensor("ebr", [P, D], f32, kind="ExternalInput")
    ngr = nc.dram_tensor("ngr", [P, D], f32, kind="ExternalInput")
    nbr = nc.dram_tensor("nbr", [P, D], f32, kind="ExternalInput")
    outp = nc.dram_tensor("outp", [N_SH, D], f32, kind="ExternalOutput")

    caps = [int(t) * 128 for t in T_seq]
    win_off = np.concatenate([[0], np.cumsum(caps)])[:-1]
    n_win = len(T_seq) if limit_windows is None else limit_windows

    with tile.TileContext(nc) as tc:
        with (
            tc.tile_pool(name="singles", bufs=1) as singles,
            tc.tile_pool(name="streams", bufs=3) as streams,
            tc.tile_pool(name="work", bufs=4) as work,
            tc.tile_pool(name="hbuf", bufs=6) as hbuf,
            tc.tile_pool(name="ph", bufs=3, space="PSUM") as ph,
            tc.tile_pool(name="pp", bufs=3, space="PSUM") as pp,
            tc.tile_pool(name="pagg", bufs=2, space="PSUM") as pagg,
        ):
            # ---- constants / weights
            w1s = singles.tile([P, 3, HID], f32r)
            nc.sync.dma_start(out=w1s, in_=ew1.rearrange("(c p) h -> p c h", p=P))
            w2s = singles.tile([P, 2, D], f32)
            nc.sync.dma_start(out=w2s, in_=ew2.rearrange("(c p) d -> p c d", p=P))
            nw1s = singles.tile([P, 2, HID], f32)
            nc.sync.dma_start(out=nw1s, in_=nw1.rearrange("(c p) h -> p c h", p=P))
            nw2s = singles.tile([P, 2, D], f32)
            nc.sync.dma_start(out=nw2s, in_=nw2.rearrange("(c p) d -> p c d", p=P))
            eb1s = singles.tile([P, 2], f32)
            nc.sync.dma_start(out=eb1s, in_=eb1.rearrange("(c p) -> p c", p=P))
            nb1s = singles.tile([P, 2], f32)
            nc.sync.dma_start(out=nb1s, in_=nb1.rearrange("(c p) -> p c", p=P))
            eb2s = singles.tile([P, D], f32)
            nc.sync.dma_start(out=eb2s, in_=eb2r[:])
            nb2s = singles.tile([P, D], f32)
            nc.sync.dma_start(out=nb2s, in_=nb2r[:])
            iotas = singles.tile([P, P], f32)
            nc.sync.dma_start(out=iotas, in_=iota[:])
            epss = singles.tile([P, 1], f32)
            nc.vector.memset(epss, EPS)
            egs = ebs = ngs = nbs = None
            if not trivial_eln:
                egs = singles.tile([P, D], f32)
                nc.sync.dma_start(out=egs, in_=egr[:])
                ebs = singles.tile([P, D], f32)
                nc.sync.dma_start(out=ebs, in_=ebr[:])
            if not trivial_nln:
                ngs = singles.tile([P, D], f32)
                nc.sync.dma_start(out=ngs, in_=ngr[:])
                nbs = singles.tile([P, D], f32)
                nc.sync.dma_start(out=nbs, in_=nbr[:])

            def layer_norm(x, g, b):
                # x: SBUF [128, D]; in-place LN along free dim
                st = work.tile([P, 6], f32, tag="st")
                nc.vector.bn_stats(st, x)
                mv = work.tile([P, 2], f32, tag="mv")
                nc.vector.bn_aggr(mv, st)
                rstd = work.tile([P, 1], f32, tag="rstd")
                nc.scalar.activation(out=rstd, in_=mv[:, 1:2], func=AF.Sqrt,
                                     bias=epss, scale=1.0)
                nc.vector.reciprocal(rstd, rstd)
                nc.vector.tensor_scalar(
                    out=x, in0=x, scalar1=mv[:, 0:1], scalar2=rstd,
                    op0=OP.subtract, op1=OP.mult)
                if g is not None:
                    nc.vector.tensor_tensor(out=x, in0=x, in1=g, op=OP.mult)
                    nc.vector.tensor_tensor(out=x, in0=x, in1=b, op=OP.add)

            aggb = None
            for w in range(n_win):
                T = int(T_seq[w])
                ET = T * 128
                off = int(win_off[w])
                sw_i = w % SW

                e0t = streams.tile([P, 512], f32r, tag="e0")
                nc.sync.dma_start(out=e0t[:, :ET], in_=e0T[:, off : off + ET])
                e1t = streams.tile([P, 512], f32r, tag="e1")
                nc.sync.dma_start(out=e1t[:, :ET], in_=e1T[:, off : off + ET])
                e2t = streams.tile([P, 512], f32r, tag="e2")
                nc.sync.dma_start(out=e2t[:, :ET], in_=e2T[:, off : off + ET])
                dlt = streams.tile([P, 4], f32, tag="dl")
                nc.sync.dma_start(
                    out=dlt[:, :T],
                    in_=dlf[off : off + ET].rearrange("(t p) -> p t", p=P))

                # edge L1: H[hc] [128h, ET]
                hts = []
                for hc in range(2):
                    hp = ph.tile([P, 512], f32, tag="h512")
                    for kc, srct in enumerate((e0t, e1t, e2t)):
                        nc.tensor.matmul(
                            hp[:, :ET],
                            lhsT=w1s[:, kc, hc * P : (hc + 1) * P],
                            rhs=srct[:, :ET],
                            start=(kc == 0), stop=(kc == 2))
                    hs = hbuf.tile([P, 512], f32, tag="hs")
                    nc.scalar.activation(out=hs[:, :ET], in_=hp[:, :ET],
                                         func=AF.Silu, bias=eb1s[:, hc : hc + 1])
                    hts.append(hs)

                aggp = pagg.tile([P, P], f32, tag="aggT")
                for t in range(T):
                    sl = slice(t * P, (t + 1) * P)
                    ef2p = pp.tile([P, P], f32, tag="p128")
                    for hc in range(2):
                        nc.tensor.matmul(
                            ef2p,
                            lhsT=hts[hc][:, sl],
                            rhs=w2s[:, hc, :],
                            start=(hc == 0), stop=(hc == 1))
                    ef2s = work.tile([P, D], f32, tag="ef2")
                    nc.vector.tensor_tensor(out=ef2s, in0=ef2p, in1=eb2s, op=OP.add)
                    layer_norm(ef2s, egs, ebs)
                    oh = work.tile([P, P], f32, tag="oh")
                    nc.vector.tensor_tensor(
                        out=oh, in0=dlt[:, t : t + 1].to_broadcast([P, P]),
                        in1=iotas, op=OP.is_equal)
                    nc.tensor.matmul(
                        aggp, lhsT=ef2s, rhs=oh,
                        start=(t == 0), stop=(t == T - 1))

                if sw_i == 0:
                    aggb = hbuf.tile([P, 512], f32, tag="aggb")
                nc.scalar.copy(out=aggb[:, sw_i * P : (sw_i + 1) * P], in_=aggp)

                # ---- node stage every SW windows
                if sw_i == SW - 1:
                    sw = w // SW
                    nsl = slice(sw * 512, (sw + 1) * 512)
                    gt = streams.tile([P, 512], f32, tag="gt")
                    nc.gpsimd.dma_start(out=gt, in_=gridT[:, nsl])
                    h2s = []
                    for hc in range(2):
                        h2p = ph.tile([P, 512], f32, tag="h512")
                        nc.tensor.matmul(
                            h2p, lhsT=nw1s[:, 0, hc * P : (hc + 1) * P],
                            rhs=aggb, start=True, stop=False)
                        nc.tensor.matmul(
                            h2p, lhsT=nw1s[:, 1, hc * P : (hc + 1) * P],
                            rhs=gt, start=False, stop=True)
                        h2 = hbuf.tile([P, 512], f32, tag="hs")
                        nc.scalar.activation(out=h2, in_=h2p, func=AF.Silu,
                                             bias=nb1s[:, hc : hc + 1])
                        h2s.append(h2)
                    for nt in range(4):
                        sl = slice(nt * P, (nt + 1) * P)
                        o2p = pp.tile([P, P], f32, tag="p128")
                        for hc in range(2):
                            nc.tensor.matmul(
                                o2p, lhsT=h2s[hc][:, sl],
                                rhs=nw2s[:, hc, :],
                                start=(hc == 0), stop=(hc == 1))
                        o2s = work.tile([P, D], f32, tag="o2")
                        nc.vector.tensor_tensor(out=o2s, in0=o2p, in1=nb2s, op=OP.add)
                        layer_norm(o2s, ngs, nbs)
                        rows = slice(sw * 512 + nt * P, sw * 512 + (nt + 1) * P)
                        gr = work.tile([P, D], f32, tag="gr")
                        nc.gpsimd.dma_start(out=gr, in_=grid_res[rows, :])
                        nc.vector.tensor_tensor(out=o2s, in0=o2s, in1=gr, op=OP.add)
                        nc.gpsimd.dma_start(out=outp[rows, :], in_=o2s)

    nc.finalize()
    return nc


# ----------------------------------------------------------------- entrypoint

def kernel(**inputs):
    import os

    from concourse.bass_utils import run_bass_kernel_spmd

    trace = bool(int(os.environ.get("KERNEL_TRACE", "0")))
    limit = os.environ.get("KERNEL_LIMIT_WINDOWS")
    limit = int(limit) if limit else None

    import time as _time
    _t0 = _time.time()
    T_seq, C, cores, unperm = _prepare(inputs)
    print(f"prep: {_time.time()-_t0:.1f}s", flush=True)

    eg = np.asarray(inputs["eg"], np.float32)
    ebeta = np.asarray(inputs["ebeta"], np.float32)
    ng = np.asarray(inputs["ng"], np.float32)
    nbeta = np.asarray(inputs["nbeta"], np.float32)
    trivial_eln = bool(np.all(eg == 1.0) and np.all(ebeta == 0.0))
    trivial_nln = bool(np.all(ng == 1.0) and np.all(nbeta == 0.0))

    _t0 = _time.time()
    nc = _build_program(T_seq, C, trivial_eln, trivial_nln,
                        limit_windows=limit)
    print(f"build: {_time.time()-_t0:.1f}s", flush=True)

    shared = dict(
        ew1=np.ascontiguousarray(inputs["eW1"], dtype=np.float32),
        ew2=np.ascontiguousarray(inputs["eW2"], dtype=np.float32),
        nw1=np.ascontiguousarray(inputs["nW1"], dtype=np.float32),
        nw2=np.ascontiguousarray(inputs["nW2"], dtype=np.float32),
        eb1=np.ascontiguousarray(inputs["eb1"], dtype=np.float32),
        nb1=np.ascontiguousarray(inputs["nb1"], dtype=np.float32),
        eb2r=np.ascontiguousarray(
            np.broadcast_to(np.asarray(inputs["eb2"], np.float32), (P, D))),
        nb2r=np.ascontiguousarray(
            np.broadcast_to(np.asarray(inputs["nb2"], np.float32), (P, D))),
        iota=np.ascontiguousarray(
            np.broadcast_to(np.arange(P, dtype=np.float32), (P, P))),
        egr=np.ascontiguousarray(np.broadcast_to(eg, (P, D))),
        ebr=np.ascontiguousarray(np.broadcast_to(ebeta, (P, D))),
        ngr=np.ascontiguousarray(np.broadcast_to(ng, (P, D))),
        nbr=np.ascontiguousarray(np.broadcast_to(nbeta, (P, D))),
    )
    in_maps = []
    for c in range(N_CORES):
        m = dict(shared)
        m.update(cores[c])
        in_maps.append(m)

    _t0 = _time.time()
    res = run_bass_kernel_spmd(nc, in_maps, core_ids=list(range(N_CORES)),
                               trace=trace)
    print(f"compile+exec: {_time.time()-_t0:.1f}s", flush=True)
    if res.exec_time_ns is not None:
        print(f"HW exec time: {res.exec_time_ns} ns", flush=True)
    full = np.concatenate([res.results[c]["outp"] for c in range(N_CORES)], axis=0)
    return np.ascontiguousarray(full[unperm])

